# revision 21
# baseline (speedup 1.0000x reference)
"""AdaptiveMixing Trainium2 kernel (8 NeuronCores, pure data parallel).

Math: out[b,s] = sum_k softmax(ada_mask[b,s])[k] * xpad[b, s+k-10]  (K=21)

Key idea: with S=128 on SBUF partitions and H*W on the free dim, the
spectral sliding-window reduction is a single 128x128 banded matmul
per free-dim tile:
    out[s_o, f] = sum_{s} Wb[s_o, s] * x[s, f],
    Wb[s_o, s] = w[s_o, s - s_o + 10] for |s - s_o| <= 10 else 0
so the TensorEngine does all cross-partition movement:
    out = lhsT.T @ x with lhsT[s, s_o] = Wb[s_o, s].

Band build (on device, off the DMA path):
  1. dstack[p,k,f] = 1 if f == p + k - 10 else 0   (gpsimd affine_select,
     no input dependency -- runs at kernel start)
  2. softmax numerator wexp = exp(mask - max) (+ row sums via accum_out);
     normalization is folded into the PSUM->SBUF copies as a per-partition
     reciprocal multiply.
  3. DW = dstack * wexp (broadcast along f)        (one DVE op)
  4. E'[p,f] = sum_k DW[p,k,f]                     (DVE reduce over k)
     E'[s_o, s_src] = wexp[s_o, s_src - s_o + 10]
  5. band = E'.T via PE transpose                  (lhsT for the big matmuls)

Sharding (host side): core i <- batch b = i//2, H-half h = i%2.
Each core handles x[b, :, h*64:(h+1)*64, :] as a (128, 8192) slab.
No communication needed.
"""

import os

import numpy as np

B, S, H, W = 4, 128, 128, 128
K = 21
PAD = 10
N_CORES = 8
H_SPLIT = 2
HS = H // H_SPLIT          # 64 rows of H per core
FREE = HS * W              # 8192
CHUNK = 1024               # free-dim elements per DMA chunk
MM_N = 512                 # matmul free dim per instruction

# fp32 PE matmul runs at 4 cycles/col; float32r (same 4-byte data) runs at
# 1 cycle/col for free >= 256. Toggle for A/B testing.
USE_F32R = os.environ.get("KERNEL_F32", "") != "1"

_COMPILED = {}


def _install_light_tail():
    """Tile's tail is drain + barrier + sem clears + barrier. The final
    all-engine barrier only fences the gpsimd-issued sem/dma clears, which
    retire on their own before the NEFF can complete -- drop it (~2-4us)."""
    import concourse.tile as tile

    if getattr(tile.TileContext, "_light_tail", False):
        return

    def _drain_and_barrier(self, tick_clock, wait_clock):
        drain_inst = self.nc.sync.drain()
        wait_clock.add_sem_waits(
            drain_inst.ins,
            _scoped_clock({None: tick_clock.global_clock}),
        )
        self.nc.all_engine_barrier()
        assert self.sems is not None
        popped = self.nc._tile_sem_poison_stack.pop()
        assert popped is self._sem_poison
        self.nc.clear_and_free_semaphores(list(self.sems.allocated().values()))

    import bass_rust as _bass_rust

    def _scoped_clock(d):
        return _bass_rust.ScopedClock(d)

    tile.TileContext._drain_and_barrier = _drain_and_barrier
    tile.TileContext._light_tail = True


def _build_nc():
    import concourse.bass as bass
    import concourse.mybir as mybir
    import concourse.tile as tile
    from concourse import bacc

    _install_light_tail()

    f32 = mybir.dt.float32
    mm_dt = mybir.dt.float32r if USE_F32R else f32
    # Bacc (not Bass): its compile() legalizes sem waits to <=1 per
    # instruction, which this walrus requires.
    nc = bacc.Bacc()
    x_d = nc.declare_dram_parameter("x", [S, FREE], mm_dt, isOutput=False)
    m_d = nc.declare_dram_parameter("mask", [S, K], f32, isOutput=False)
    o_d = nc.declare_dram_parameter("out", [S, FREE], f32, isOutput=True)

    with tile.TileContext(nc) as tc:
        with (
            tc.tile_pool(name="singles", bufs=1) as singles,
            tc.tile_pool(name="xin", bufs=8) as xin,
            tc.tile_pool(name="oout", bufs=6) as oout,
            tc.tile_pool(name="psum", bufs=6, space="PSUM") as psum,
            tc.tile_pool(name="psumT", bufs=1, space="PSUM") as psumT,
        ):
            # ---- preload the Exp activation table off the critical path ----
            warm = singles.tile([S, 1], f32)
            nc.vector.memset(warm[:], 0.0)
            nc.scalar.activation(
                out=warm[:], in_=warm[:], func=mybir.ActivationFunctionType.Exp
            )

            # ---- shifted-identity stack: no input deps, starts immediately.
            # Layout (p, f, k) with k contiguous so the k-reduction below is a
            # fast contiguous DVE reduce.
            ident = singles.tile([S, S], f32)
            nc.vector.memset(ident[:], 0.0)
            nc.gpsimd.affine_select(
                out=ident[:],
                in_=ident[:],
                compare_op=mybir.AluOpType.not_equal,
                fill=1.0,
                base=0,
                pattern=[[-1, S]],
                channel_multiplier=1,
            )
            dstack = singles.tile([S, K, S], mybir.dt.bfloat16)
            nc.gpsimd.memset(dstack[:], 0.0)
            nc.gpsimd.affine_select(
                out=dstack[:],
                in_=dstack[:],
                compare_op=mybir.AluOpType.not_equal,
                fill=1.0,
                base=-PAD,
                # affine(p,k,f) = p + k - f - PAD ; == 0 -> fill 1.0
                pattern=[[1, K], [-1, S]],
                channel_multiplier=1,
            )

            # ---- softmax numerator (normalization folded into epilogue) ----
            # mask DMA rides the (otherwise idle) ACT DGE ring so it lands
            # ahead of the big x transfers on the SP ring.
            mask_t = singles.tile([S, K], f32)
            nc.scalar.dma_start(out=mask_t[:], in_=m_d[:])

            mx = singles.tile([S, 1], f32)
            nc.vector.reduce_max(mx[:], mask_t[:], axis=mybir.AxisListType.X)
            negmx = singles.tile([S, 1], f32)
            nc.vector.tensor_scalar_mul(negmx[:], mx[:], -1.0)

            wexp = singles.tile([S, K], f32)
            wsum = singles.tile([S, 1], f32)
            nc.scalar.activation(
                out=wexp[:],
                in_=mask_t[:],
                func=mybir.ActivationFunctionType.Exp,
                bias=negmx[:],
                scale=1.0,
                accum_out=wsum[:],
            )
            rsum = singles.tile([S, 1], f32)
            nc.vector.reciprocal(rsum[:], wsum[:])

            # ---- banded weight matrix ----
            # multiply + k-reduce split across DVE (2/3) and GpSimd (1/3);
            # gpsimd's 2-input ops run ~2x slower, hence the asymmetry.
            FSPLIT = 84
            dw = singles.tile([S, K, S], f32)
            eprime = singles.tile([S, S], f32)
            for eng, flo, fhi in (
                (nc.vector, 0, FSPLIT),
                (nc.gpsimd, FSPLIT, S),
            ):
                eng.tensor_tensor(
                    dw[:, :, flo:fhi],
                    dstack[:, :, flo:fhi],
                    wexp[:, :, None].to_broadcast((S, K, fhi - flo)),
                    mybir.AluOpType.mult,
                )
            for flo, fhi in ((0, FSPLIT), (FSPLIT, S)):
                nc.vector.reduce_sum(
                    eprime[:, flo:fhi],
                    dw[:, :, flo:fhi].rearrange("p k f -> p f k"),
                    axis=mybir.AxisListType.X,
                )
            band_ps = psumT.tile([S, S], f32)
            nc.tensor.transpose(band_ps[:], eprime[:], ident[:])
            band = singles.tile([S, S], mm_dt)
            nc.vector.tensor_copy(out=band[:], in_=band_ps[:])

            # ---- stream x through the banded matmul ----
            n_chunks = FREE // CHUNK
            mm_per_chunk = CHUNK // MM_N
            for c in range(n_chunks):
                xt = xin.tile([S, CHUNK], mm_dt)
                nc.sync.dma_start(
                    out=xt[:], in_=x_d[:, c * CHUNK : (c + 1) * CHUNK]
                )
                ot = oout.tile([S, CHUNK], f32)
                for j in range(mm_per_chunk):
                    ps = psum.tile([S, MM_N], f32)
                    nc.tensor.matmul(
                        ps[:],
                        lhsT=band[:],
                        rhs=xt[:, j * MM_N : (j + 1) * MM_N],
                        start=True,
                        stop=True,
                    )
                    # epilogue: copy + softmax denominator (per-partition),
                    # alternating DVE / ScalarE to halve the epilogue wall
                    oslice = ot[:, j * MM_N : (j + 1) * MM_N]
                    if (c * mm_per_chunk + j) % 2 == 0:
                        nc.vector.tensor_scalar_mul(oslice, ps[:], rsum[:])
                    else:
                        nc.scalar.activation(
                            out=oslice,
                            in_=ps[:],
                            func=mybir.ActivationFunctionType.Copy,
                            bias=0.0,
                            scale=rsum[:],
                        )
                nc.sync.dma_start(
                    out=o_d[:, c * CHUNK : (c + 1) * CHUNK], in_=ot[:]
                )

    nc.finalize()
    return nc


def _get_compiled():
    if "nc" not in _COMPILED:
        _COMPILED["nc"] = _build_nc()
    return _COMPILED["nc"]


def _shard_inputs(x, ada_mask):
    in_maps = []
    for i in range(N_CORES):
        b, h = divmod(i, H_SPLIT)
        xs = np.ascontiguousarray(
            x[b, :, h * HS : (h + 1) * HS, :].reshape(S, FREE)
        ).astype(np.float32, copy=False)
        ms = np.ascontiguousarray(ada_mask[b]).astype(np.float32, copy=False)
        in_maps.append({"x": xs, "mask": ms})
    return in_maps


def _run(x, ada_mask, trace=False, tmpdir=None):
    from concourse.bass_utils import run_bass_kernel_spmd

    nc = _get_compiled()
    in_maps = _shard_inputs(x, ada_mask)
    res = run_bass_kernel_spmd(
        nc,
        in_maps,
        core_ids=list(range(N_CORES)),
        trace=trace,
        tmpdir=tmpdir,
    )
    out = np.empty((B, S, H, W), dtype=np.float32)
    for i in range(N_CORES):
        b, h = divmod(i, H_SPLIT)
        out[b, :, h * HS : (h + 1) * HS, :] = res.results[i]["out"].reshape(S, HS, W)
    return out, res


def kernel(x, ada_mask):
    x = np.asarray(x)
    ada_mask = np.asarray(ada_mask)
    out, _ = _run(x, ada_mask, trace=False)
    return out


def kernel_traced(x, ada_mask, tmpdir=None):
    """Correctness + profile run: returns (out, BassKernelResults)."""
    return _run(np.asarray(x), np.asarray(ada_mask), trace=True, tmpdir=tmpdir)


# revision 22
# speedup vs baseline: 1.1474x; 1.1474x over previous
"""AdaptiveMixing Trainium2 kernel (8 NeuronCores, pure data parallel).

Math: out[b,s] = sum_k softmax(ada_mask[b,s])[k] * xpad[b, s+k-10]  (K=21)

Key idea: with S=128 on SBUF partitions and H*W on the free dim, the
spectral sliding-window reduction is a single 128x128 banded matmul
per free-dim tile:
    out[s_o, f] = sum_{s} Wb[s_o, s] * x[s, f],
    Wb[s_o, s] = w[s_o, s - s_o + 10] for |s - s_o| <= 10 else 0
so the TensorEngine does all cross-partition movement:
    out = lhsT.T @ x with lhsT[s, s_o] = Wb[s_o, s].

Band build (on device, off the DMA path):
  1. dstack[p,k,f] = 1 if f == p + k - 10 else 0   (gpsimd affine_select,
     no input dependency -- runs at kernel start)
  2. softmax numerator wexp = exp(mask - max) (+ row sums via accum_out);
     normalization is folded into the PSUM->SBUF copies as a per-partition
     reciprocal multiply.
  3. DW = dstack * wexp (broadcast along f)        (one DVE op)
  4. E'[p,f] = sum_k DW[p,k,f]                     (DVE reduce over k)
     E'[s_o, s_src] = wexp[s_o, s_src - s_o + 10]
  5. band = E'.T via PE transpose                  (lhsT for the big matmuls)

Sharding (host side): core i <- batch b = i//2, H-half h = i%2.
Each core handles x[b, :, h*64:(h+1)*64, :] as a (128, 8192) slab.
No communication needed.
"""

import os

import numpy as np

B, S, H, W = 4, 128, 128, 128
K = 21
PAD = 10
N_CORES = 8
H_SPLIT = 2
HS = H // H_SPLIT          # 64 rows of H per core
FREE = HS * W              # 8192
CHUNK = 1024               # free-dim elements per DMA chunk
MM_N = 512                 # matmul free dim per instruction

# fp32 PE matmul runs at 4 cycles/col; float32r (same 4-byte data) runs at
# 1 cycle/col for free >= 256. Toggle for A/B testing.
USE_F32R = os.environ.get("KERNEL_F32", "") != "1"

_COMPILED = {}


def _install_light_tail():
    """Tile's tail is drain + barrier + sem clears + barrier. The final
    all-engine barrier only fences the gpsimd-issued sem/dma clears, which
    retire on their own before the NEFF can complete -- drop it (~2-4us)."""
    import concourse.tile as tile

    if getattr(tile.TileContext, "_light_tail", False):
        return

    def _drain_and_barrier(self, tick_clock, wait_clock):
        drain_inst = self.nc.sync.drain()
        wait_clock.add_sem_waits(
            drain_inst.ins,
            _scoped_clock({None: tick_clock.global_clock}),
        )
        self.nc.all_engine_barrier()
        assert self.sems is not None
        popped = self.nc._tile_sem_poison_stack.pop()
        assert popped is self._sem_poison
        self.nc.clear_and_free_semaphores(list(self.sems.allocated().values()))

    import bass_rust as _bass_rust

    def _scoped_clock(d):
        return _bass_rust.ScopedClock(d)

    tile.TileContext._drain_and_barrier = _drain_and_barrier
    tile.TileContext._light_tail = True


def _build_nc():
    import concourse.bass as bass
    import concourse.mybir as mybir
    import concourse.tile as tile
    from concourse import bacc

    _install_light_tail()

    f32 = mybir.dt.float32
    mm_dt = mybir.dt.float32r if USE_F32R else f32
    # Bacc (not Bass): its compile() legalizes sem waits to <=1 per
    # instruction, which this walrus requires.
    nc = bacc.Bacc()
    x_d = nc.declare_dram_parameter("x", [S, FREE], mm_dt, isOutput=False)
    m_d = nc.declare_dram_parameter("mask", [S, K], f32, isOutput=False)
    o_d = nc.declare_dram_parameter("out", [S, FREE], f32, isOutput=True)

    with tile.TileContext(nc) as tc:
        with (
            tc.tile_pool(name="singles", bufs=1) as singles,
            tc.tile_pool(name="xin", bufs=8) as xin,
            tc.tile_pool(name="oout", bufs=6) as oout,
            tc.tile_pool(name="psum", bufs=6, space="PSUM") as psum,
            tc.tile_pool(name="psumT", bufs=1, space="PSUM") as psumT,
        ):
            # ---- preload the Exp activation table off the critical path ----
            warm = singles.tile([S, 1], f32)
            nc.vector.memset(warm[:], 0.0)
            nc.scalar.activation(
                out=warm[:], in_=warm[:], func=mybir.ActivationFunctionType.Exp
            )

            # ---- shifted-identity stack: no input deps, starts immediately.
            # Layout (p, f, k) with k contiguous so the k-reduction below is a
            # fast contiguous DVE reduce.
            ident = singles.tile([S, S], f32)
            nc.vector.memset(ident[:], 0.0)
            nc.gpsimd.affine_select(
                out=ident[:],
                in_=ident[:],
                compare_op=mybir.AluOpType.not_equal,
                fill=1.0,
                base=0,
                pattern=[[-1, S]],
                channel_multiplier=1,
            )
            dstack = singles.tile([S, K, S], f32)
            nc.gpsimd.memset(dstack[:], 0.0)
            nc.gpsimd.affine_select(
                out=dstack[:],
                in_=dstack[:],
                compare_op=mybir.AluOpType.not_equal,
                fill=1.0,
                base=-PAD,
                # affine(p,k,f) = p + k - f - PAD ; == 0 -> fill 1.0
                pattern=[[1, K], [-1, S]],
                channel_multiplier=1,
            )

            # ---- softmax numerator (normalization folded into epilogue) ----
            mask_t = singles.tile([S, K], f32)
            nc.sync.dma_start(out=mask_t[:], in_=m_d[:])

            mx = singles.tile([S, 1], f32)
            nc.vector.reduce_max(mx[:], mask_t[:], axis=mybir.AxisListType.X)
            negmx = singles.tile([S, 1], f32)
            nc.vector.tensor_scalar_mul(negmx[:], mx[:], -1.0)

            wexp = singles.tile([S, K], f32)
            wsum = singles.tile([S, 1], f32)
            nc.scalar.activation(
                out=wexp[:],
                in_=mask_t[:],
                func=mybir.ActivationFunctionType.Exp,
                bias=negmx[:],
                scale=1.0,
                accum_out=wsum[:],
            )
            rsum = singles.tile([S, 1], f32)
            nc.vector.reciprocal(rsum[:], wsum[:])

            # ---- banded weight matrix ----
            # multiply + k-reduce split across DVE (2/3) and GpSimd (1/3);
            # gpsimd's 2-input ops run ~2x slower, hence the asymmetry.
            dw = singles.tile([S, K, S], f32)
            nc.vector.tensor_tensor(
                dw[:],
                dstack[:],
                wexp[:, :, None].to_broadcast((S, K, S)),
                mybir.AluOpType.mult,
            )
            eprime = singles.tile([S, S], f32)
            nc.vector.reduce_sum(
                eprime[:],
                dw[:].rearrange("p k f -> p f k"),
                axis=mybir.AxisListType.X,
            )
            band_ps = psumT.tile([S, S], f32)
            nc.tensor.transpose(band_ps[:], eprime[:], ident[:])
            band = singles.tile([S, S], mm_dt)
            nc.vector.tensor_copy(out=band[:], in_=band_ps[:])

            # ---- stream x through the banded matmul ----
            n_chunks = FREE // CHUNK
            mm_per_chunk = CHUNK // MM_N
            for c in range(n_chunks):
                xt = xin.tile([S, CHUNK], mm_dt)
                nc.sync.dma_start(
                    out=xt[:], in_=x_d[:, c * CHUNK : (c + 1) * CHUNK]
                )
                ot = oout.tile([S, CHUNK], f32)
                for j in range(mm_per_chunk):
                    ps = psum.tile([S, MM_N], f32)
                    nc.tensor.matmul(
                        ps[:],
                        lhsT=band[:],
                        rhs=xt[:, j * MM_N : (j + 1) * MM_N],
                        start=True,
                        stop=True,
                    )
                    # epilogue: copy + softmax denominator (per-partition),
                    # alternating DVE / ScalarE to halve the epilogue wall
                    oslice = ot[:, j * MM_N : (j + 1) * MM_N]
                    if (c * mm_per_chunk + j) % 2 == 0:
                        nc.vector.tensor_scalar_mul(oslice, ps[:], rsum[:])
                    else:
                        nc.scalar.activation(
                            out=oslice,
                            in_=ps[:],
                            func=mybir.ActivationFunctionType.Copy,
                            bias=0.0,
                            scale=rsum[:],
                        )
                nc.sync.dma_start(
                    out=o_d[:, c * CHUNK : (c + 1) * CHUNK], in_=ot[:]
                )

    nc.finalize()
    return nc


def _get_compiled():
    if "nc" not in _COMPILED:
        _COMPILED["nc"] = _build_nc()
    return _COMPILED["nc"]


def _shard_inputs(x, ada_mask):
    in_maps = []
    for i in range(N_CORES):
        b, h = divmod(i, H_SPLIT)
        xs = np.ascontiguousarray(
            x[b, :, h * HS : (h + 1) * HS, :].reshape(S, FREE)
        ).astype(np.float32, copy=False)
        ms = np.ascontiguousarray(ada_mask[b]).astype(np.float32, copy=False)
        in_maps.append({"x": xs, "mask": ms})
    return in_maps


def _run(x, ada_mask, trace=False, tmpdir=None):
    from concourse.bass_utils import run_bass_kernel_spmd

    nc = _get_compiled()
    in_maps = _shard_inputs(x, ada_mask)
    res = run_bass_kernel_spmd(
        nc,
        in_maps,
        core_ids=list(range(N_CORES)),
        trace=trace,
        tmpdir=tmpdir,
    )
    out = np.empty((B, S, H, W), dtype=np.float32)
    for i in range(N_CORES):
        b, h = divmod(i, H_SPLIT)
        out[b, :, h * HS : (h + 1) * HS, :] = res.results[i]["out"].reshape(S, HS, W)
    return out, res


def kernel(x, ada_mask):
    x = np.asarray(x)
    ada_mask = np.asarray(ada_mask)
    out, _ = _run(x, ada_mask, trace=False)
    return out


def kernel_traced(x, ada_mask, tmpdir=None):
    """Correctness + profile run: returns (out, BassKernelResults)."""
    return _run(np.asarray(x), np.asarray(ada_mask), trace=True, tmpdir=tmpdir)
